# revision 1
# baseline (speedup 1.0000x reference)
"""Causal self-attention Trainium2 kernel (8 NeuronCores).

Sharding: core = (batch b in {0,1}, head-group hg in {0..3}); each core owns
4 of the 16 heads (256 of the 1024 q/k/v dims) for one batch element.
Data parallel over batch, tensor parallel over heads; W_o is row-parallel so
each core emits a partial output that the host sums (+ b_o) at gather time.

Device dataflow (per core), everything in "transposed" layout so the
contraction dim always sits on SBUF partitions:
  xT [1024,2048] bf16, weights pre-transposed+cast on host.
  QT/KT [d=256, s=2048] bf16 (d on partitions, 2 blocks of 128)
  V natural [s, d] bf16 with a ones-column appended per head so the A@V
  matmul also produces the softmax denominator (row 64 of the PSUM tile).
  Scores transposed: ST[k, q] = KT_chunk.T @ QT (fp32 PSUM), causal mask
  added (-1e30) on DVE, exp on ACT -> bf16 AexpT, then A@V accumulated over
  k chunks. Normalization via reciprocal_approx_fast + gpsimd broadcast.
  Output projection yT[m, s] = WoT.T @ outT done in two d-halves: the first
  half lands while heads 2-3 still run; the second half accumulates into
  DRAM with a gpsimd accum DMA.
"""

import sys

for _p in ("/opt/trn_rl_repo",):
    if _p not in sys.path:
        sys.path.insert(0, _p)

import numpy as np
import ml_dtypes

import concourse.bass as bass
import concourse.bacc as bacc
import concourse.mybir as mybir
from concourse import tile
from concourse.bass_utils import run_bass_kernel_spmd

P = 128
S = 2048  # sequence length
D = 1024  # d_model
DG = 256  # dims per head-group (4 heads x 64)
DH = 64   # head dim
NHG = 4   # heads per core
EC = D // P   # 8 contraction chunks over d_model
KC = S // P   # 16 key chunks
QTW = 512     # q tile width
NQT = S // QTW  # 4 q tiles
F32 = mybir.dt.float32
BF16 = mybir.dt.bfloat16
NEG = -1.0e30
AF = mybir.ActivationFunctionType

LAST_EXEC_NS = None
LAST_RESULTS = None


def _emit(tc, aps):
    nc = tc.nc
    xt_d, wqt_d, wkt_d, wvt_d, wot_d, bq_d, bk_d, bv_d, yt_d = aps

    with (
        tc.tile_pool(name="const", bufs=1) as constp,
        tc.tile_pool(name="wpool", bufs=1) as wp,
        tc.tile_pool(name="xpool", bufs=1) as xp,
        tc.tile_pool(name="qkvp", bufs=1) as qkvp,
        tc.tile_pool(name="aep", bufs=10) as aep,
        tc.tile_pool(name="outp", bufs=1) as outp,
        tc.tile_pool(name="normp", bufs=3) as normp,
        tc.tile_pool(name="stagep", bufs=4) as stagep,
        tc.tile_pool(name="psum_st", bufs=6, space="PSUM") as psum_st,
        tc.tile_pool(name="psum_mm", bufs=2, space="PSUM") as psum_mm,
    ):
        # ---- persistent SBUF tensors ----
        mask_wide = constp.tile([P, 896], F32, name="mask_wide")
        bq_sb = constp.tile([P, 2], F32, name="bq_sb")
        bqs_sb = constp.tile([P, 2], F32, name="bqs_sb")
        bk_sb = constp.tile([P, 2], F32, name="bk_sb")
        bv1_sb = constp.tile([1, DG], F32, name="bv1_sb")
        bvb_sb = constp.tile([P, DG], F32, name="bvb_sb")

        wqt_sb = wp.tile([P, EC, DG], BF16, name="wqt_sb")
        wkt_sb = wp.tile([P, EC, DG], BF16, name="wkt_sb")
        wvt_sb = wp.tile([P, EC, DG], BF16, name="wvt_sb")
        wot_sb = wp.tile([P, 2, D], BF16, name="wot_sb")

        xt_sb = xp.tile([P, EC, S], BF16, name="xt_sb")

        qt_sb = qkvp.tile([P, 2, S], BF16, name="qt_sb")
        kt_sb = qkvp.tile([P, 2, S], BF16, name="kt_sb")
        v_sb = qkvp.tile([P, KC, NHG, DH + 1], BF16, name="v_sb")

        outt_sb = outp.tile([P, 2, S], BF16, name="outt_sb")

        # ---- constants ----
        for c in range(2):
            nc.sync.dma_start(bq_sb[:, c : c + 1], bq_d[c * P : (c + 1) * P, :])
            nc.sync.dma_start(bk_sb[:, c : c + 1], bk_d[c * P : (c + 1) * P, :])
        nc.scalar.mul(bqs_sb[:, :], bq_sb[:, :], 0.125)
        nc.sync.dma_start(bv1_sb[:, :], bv_d[:, :])
        nc.gpsimd.partition_broadcast(bvb_sb[:, :], bv1_sb[:, :], channels=P)
        # causal mask, shared across diagonal offsets: mask_wide[x, y] = 0 if
        # y - x >= 384 else -1e30; slice [384-128j : 896-128j] gives the mask
        # for a diagonal chunk with k0 = q0 + 128j.
        nc.gpsimd.memset(mask_wide[:, :], 0.0)
        nc.gpsimd.affine_select(
            out=mask_wide[:, :],
            in_=mask_wide[:, :],
            compare_op=mybir.AluOpType.is_ge,
            fill=NEG,
            base=-384,
            pattern=[[1, 896]],
            channel_multiplier=-1,
        )
        # ones column in V for the fused softmax denominator
        nc.vector.memset(v_sb[:, :, :, DH : DH + 1], 1.0)

        # ---- input DMAs (xt first: QKV streams on it chunk by chunk) ----
        for ec in range(EC):
            nc.sync.dma_start(xt_sb[:, ec, :], xt_d[ec * P : (ec + 1) * P, :])
            nc.sync.dma_start(wqt_sb[:, ec, :], wqt_d[ec * P : (ec + 1) * P, :])
            nc.sync.dma_start(wkt_sb[:, ec, :], wkt_d[ec * P : (ec + 1) * P, :])
            nc.sync.dma_start(wvt_sb[:, ec, :], wvt_d[ec * P : (ec + 1) * P, :])
        for dc in range(2):
            nc.sync.dma_start(wot_sb[:, dc, :], wot_d[dc * P : (dc + 1) * P, :])

        # ---- QKV projections ----
        # QT/KT: ec-outer over 4 parked PSUM tiles -> streams on the xt DMAs
        # and reuses each weight chunk across the 4 s-tiles.
        def qk_block(w_sb, dst_sb, db, bias_ap, scale):
            ps = [
                psum_st.tile([P, QTW], F32, name=f"pqk{t}", tag="st")
                for t in range(NQT)
            ]
            for ec in range(EC):
                for t in range(NQT):
                    nc.tensor.matmul(
                        ps[t][:, :],
                        w_sb[:, ec, db * P : (db + 1) * P],
                        xt_sb[:, ec, t * QTW : (t + 1) * QTW],
                        start=(ec == 0),
                        stop=(ec == EC - 1),
                    )
            for t in range(NQT):
                nc.scalar.activation(
                    dst_sb[:, db, t * QTW : (t + 1) * QTW],
                    ps[t][:, :],
                    AF.Identity,
                    bias=bias_ap,
                    scale=scale,
                )

        for db in range(2):
            # Q scaled by 1/sqrt(dh)=0.125 here (bias pre-scaled too)
            qk_block(wqt_sb, qt_sb, db, bqs_sb[:, db : db + 1], 0.125)
            qk_block(wkt_sb, kt_sb, db, bk_sb[:, db : db + 1], 1.0)

        # V natural [s-chunk 128, d 256]: xT_chunk (stationary) vs wvT (moving)
        for sc in range(KC):
            pv = psum_mm.tile([P, QTW], F32, name="pv", tag="av")
            for ec in range(EC):
                nc.tensor.matmul(
                    pv[:, :DG],
                    xt_sb[:, ec, sc * P : (sc + 1) * P],
                    wvt_sb[:, ec, :],
                    start=(ec == 0),
                    stop=(ec == EC - 1),
                )
            nc.vector.tensor_add(
                v_sb[:, sc, :, 0:DH],
                pv[:, :DG].rearrange("p (h d) -> p h d", h=NHG),
                bvb_sb[:, :].rearrange("p (h d) -> p h d", h=NHG),
            )

        # ---- attention: per head, k-chunk-outer / q-tile-inner ----
        # For chunk c the valid q tiles are t >= c//4; the KT chunk and the
        # V chunk are each loaded as stationary once per (head, c) and the
        # 4 per-tile A@V accumulators live in the 4 "mm" PSUM slots for the
        # whole head. ST matmuls for chunk c+1 are emitted before the A@V
        # matmuls of chunk c so the PE always has independent work while ACT
        # chews through the exps.
        def norm_dispatch(h, t, av):
            # Free the PSUM accumulator fast: unnormalized copy out + denom
            # row extract; the reciprocal/broadcast/scale chain then runs off
            # the critical path (normalizing outt in place in SBUF).
            dc, po = divmod(h, 2)
            tq = slice(t * QTW, (t + 1) * QTW)
            den = normp.tile([1, QTW], F32, name="den", tag="den")
            nc.vector.tensor_copy(den[:, :], av[DH : DH + 1, :])
            if po == 0:
                dst = outt_sb[0:DH, dc, tq]
                nc.vector.tensor_copy(dst, av[0:DH, :])
            else:
                dst = normp.tile([DH, QTW], BF16, name="odd", tag="odd")
                nc.vector.tensor_copy(dst[:, :], av[0:DH, :])
            # reciprocal with all 128 lanes: DMA-reshape [1,512]->[128,4]
            denP = normp.tile([P, 4], F32, name="denP", tag="denP")
            nc.sync.dma_start(denP[:, :], den[:, :])
            recP = normp.tile([P, 4], F32, name="recP", tag="recP")
            nc.vector.reciprocal(recP[:, :], denP[:, :])
            rec = normp.tile([1, QTW], F32, name="rec", tag="rec")
            nc.sync.dma_start(rec[:, :], recP[:, :])
            bc = normp.tile([DH, QTW], F32, name="bc", tag="bc")
            nc.gpsimd.partition_broadcast(bc[:, :], rec[:, :], channels=DH)
            nc.vector.tensor_mul(dst, dst, bc[:, :])
            if po == 1:
                nc.sync.dma_start(outt_sb[DH:P, dc, tq], dst)

        def wo_tile(st4):
            # yT[:, q-tile st4] = sum_dc WoT_chunk.T @ outT_chunk; emitted as
            # soon as the last head finishes this q tile, so the output
            # projection overlaps the remaining attention work.
            for mc in range(8):
                py = psum_st.tile([P, QTW], F32, name="py", tag="st")
                for dcw in range(2):
                    nc.tensor.matmul(
                        py[:, :],
                        wot_sb[:, dcw, mc * P : (mc + 1) * P],
                        outt_sb[:, dcw, st4 * QTW : (st4 + 1) * QTW],
                        start=(dcw == 0),
                        stop=(dcw == 1),
                    )
                sg = stagep.tile([P, QTW], F32, name="sg", tag="yst")
                nc.scalar.copy(sg[:, :], py[:, :])
                nc.sync.dma_start(
                    yt_d[mc * P : (mc + 1) * P, st4 * QTW : (st4 + 1) * QTW],
                    sg[:, :],
                )

        # Two heads with the SAME partition offset (po) run as interleaved
        # streams (alternating po per instruction trips a HW hazard), one q
        # tile at a time: 2 A@V accumulators + up to 3 chunks of ST lookahead
        # (6 "st" slots) keep the PE saturated while ACT chews the exps.
        def emit_st_one(h, c, t):
            dc, po = divmod(h, 2)
            qoff = po * DH
            stp = psum_st.tile([P, QTW], F32, name="stp", tag="st")
            nc.tensor.matmul(
                stp[:, :],
                kt_sb[qoff : qoff + DH, dc, c * P : (c + 1) * P],
                qt_sb[qoff : qoff + DH, dc, t * QTW : (t + 1) * QTW],
                start=True,
                stop=True,
            )
            return stp

        for m in range(2):
            heads = (m, m + 2)
            for t in range(NQT):
                cmax = 4 * t + 4  # chunks 0 .. 4t+3
                avs = {
                    h: psum_mm.tile([P, QTW], F32, name=f"av{h}", tag="av")
                    for h in heads
                }
                sts = {(h, c): emit_st_one(h, c, t) for c in (0, 1) for h in heads}
                for c in range(cmax):
                    aes = {}
                    for h in heads:
                        if t == c // 4:
                            off = 384 - 128 * (c % 4)
                            nc.vector.tensor_add(
                                sts[(h, c)][:, :],
                                sts[(h, c)][:, :],
                                mask_wide[:, off : off + QTW],
                            )
                        ae = aep.tile([P, QTW], BF16, name="ae", tag="ae")
                        nc.scalar.activation(ae[:, :], sts[(h, c)][:, :], AF.Exp)
                        aes[h] = ae
                    if c + 2 < cmax:
                        for h in heads:
                            sts[(h, c + 2)] = emit_st_one(h, c + 2, t)
                    for h in heads:
                        nc.tensor.matmul(
                            avs[h][0 : DH + 1, :],
                            v_sb[:, c, h, :],
                            aes[h][:, :],
                            start=(c == 0),
                            stop=(c == cmax - 1),
                        )
                for h in heads:
                    norm_dispatch(h, t, avs[h])
                if m == 1:
                    wo_tile(t)


_NC_CACHE = None


def build_nc():
    global _NC_CACHE
    if _NC_CACHE is not None:
        return _NC_CACHE
    nc = bacc.Bacc("TRN2")
    xt = nc.dram_tensor("xt", [D, S], BF16, kind="ExternalInput")
    wqt = nc.dram_tensor("wqt", [D, DG], BF16, kind="ExternalInput")
    wkt = nc.dram_tensor("wkt", [D, DG], BF16, kind="ExternalInput")
    wvt = nc.dram_tensor("wvt", [D, DG], BF16, kind="ExternalInput")
    wot = nc.dram_tensor("wot", [DG, D], BF16, kind="ExternalInput")
    bq = nc.dram_tensor("bq", [DG, 1], F32, kind="ExternalInput")
    bk = nc.dram_tensor("bk", [DG, 1], F32, kind="ExternalInput")
    bv = nc.dram_tensor("bv", [1, DG], F32, kind="ExternalInput")
    yt = nc.dram_tensor("yt", [D, S], F32, kind="ExternalOutput")
    aps = tuple(h.ap() for h in (xt, wqt, wkt, wvt, wot, bq, bk, bv, yt))
    with tile.TileContext(nc) as tc:
        _emit(tc, aps)
    nc.finalize()
    _NC_CACHE = nc
    return nc


def make_in_maps(x, W_q, b_q, W_k, b_k, W_v, b_v, W_o):
    bf = ml_dtypes.bfloat16
    in_maps = []
    for core in range(8):
        b, hg = divmod(core, 4)
        sl = slice(hg * DG, (hg + 1) * DG)
        in_maps.append(
            {
                "xt": np.ascontiguousarray(np.asarray(x)[b].T.astype(bf)),
                "wqt": np.ascontiguousarray(np.asarray(W_q)[sl, :].T.astype(bf)),
                "wkt": np.ascontiguousarray(np.asarray(W_k)[sl, :].T.astype(bf)),
                "wvt": np.ascontiguousarray(np.asarray(W_v)[sl, :].T.astype(bf)),
                "wot": np.ascontiguousarray(np.asarray(W_o)[:, sl].T.astype(bf)),
                "bq": np.ascontiguousarray(
                    np.asarray(b_q)[sl].reshape(DG, 1), dtype=np.float32
                ),
                "bk": np.ascontiguousarray(
                    np.asarray(b_k)[sl].reshape(DG, 1), dtype=np.float32
                ),
                "bv": np.ascontiguousarray(
                    np.asarray(b_v)[sl].reshape(1, DG), dtype=np.float32
                ),
            }
        )
    return in_maps


def kernel(x, W_q, b_q, W_k, b_k, W_v, b_v, W_o, b_o, _trace=False):
    global LAST_EXEC_NS, LAST_RESULTS
    nc = build_nc()
    in_maps = make_in_maps(x, W_q, b_q, W_k, b_k, W_v, b_v, W_o)
    kw = {"trace": True} if _trace else {}
    res = run_bass_kernel_spmd(nc, in_maps, core_ids=list(range(8)), **kw)
    LAST_EXEC_NS = res.exec_time_ns
    LAST_RESULTS = res
    b_o = np.asarray(b_o, dtype=np.float32)
    out = np.empty((2, S, D), np.float32)
    for b in range(2):
        ysum = (
            res.results[4 * b]["yt"]
            + res.results[4 * b + 1]["yt"]
            + res.results[4 * b + 2]["yt"]
            + res.results[4 * b + 3]["yt"]
        )
        out[b] = ysum.T + b_o
    return out



# revision 2
# speedup vs baseline: 1.0347x; 1.0347x over previous
"""Causal self-attention Trainium2 kernel (8 NeuronCores).

Sharding: core = (batch b in {0,1}, head-group hg in {0..3}); each core owns
4 of the 16 heads (256 of the 1024 q/k/v dims) for one batch element.
Data parallel over batch, tensor parallel over heads; W_o is row-parallel so
each core emits a partial output that the host sums (+ b_o) at gather time.

Device dataflow (per core), everything in "transposed" layout so the
contraction dim always sits on SBUF partitions:
  xT [1024,2048] bf16, weights pre-transposed+cast on host.
  QT/KT [d=256, s=2048] bf16 (d on partitions, 2 blocks of 128)
  V natural [s, d] bf16 with a ones-column appended per head so the A@V
  matmul also produces the softmax denominator (row 64 of the PSUM tile).

Key perf structure vs the naive version:
  - Scores for two adjacent k-chunks of one head land in one 2-bank PSUM
    pair tile [128,1024]; ONE ACT Exp instruction covers both chunks
    (amortizes the ~352-cycle ACTIVATE fixed cost; ACT is the critical
    engine in the attention phase).
  - Causal masking happens AFTER the exp, as a gpsimd affine_select that
    zeroes the upper-triangle entries of the bf16 exp tile (exp can't
    overflow: scores are bounded by ~2 after the 1/8 scale). This keeps
    both DVE and ACT off the mask work.
  - t-outer / head-inner attention; the db=1 Q/K projection blocks and the
    V projection pairs are emitted interleaved into early t=0 attention so
    the exp stream starts as soon as Q/K db0 + V chunks 0-3 exist.
  - Normalization: direct DVE reciprocal on the denominator row (no DMA
    reshape round-trip), gpsimd partition_broadcast, one DVE multiply
    straight out of PSUM.
  - W_o projection per q-tile as soon as its last head is normalized;
    PSUM->SBUF evacuation on DVE (ACT stays reserved for exps).
"""

import sys

for _p in ("/opt/trn_rl_repo",):
    if _p not in sys.path:
        sys.path.insert(0, _p)

import numpy as np
import ml_dtypes

import concourse.bass as bass
import concourse.bacc as bacc
import concourse.mybir as mybir
from concourse import tile
from concourse.bass_utils import run_bass_kernel_spmd

P = 128
S = 2048  # sequence length
D = 1024  # d_model
DG = 256  # dims per head-group (4 heads x 64)
DH = 64   # head dim
NHG = 4   # heads per core
EC = D // P   # 8 contraction chunks over d_model
KC = S // P   # 16 key chunks
QTW = 512     # q tile width
NQT = S // QTW  # 4 q tiles
W2 = 2 * QTW  # pair-tile width (2 PSUM banks)
F32 = mybir.dt.float32
BF16 = mybir.dt.bfloat16
AF = mybir.ActivationFunctionType

LAST_EXEC_NS = None
LAST_RESULTS = None


def _emit(tc, aps):
    nc = tc.nc
    xt_d, wqt_d, wkt_d, wvt_d, wot_d, bq_d, bk_d, bv_d, yt_d = aps

    with (
        tc.tile_pool(name="const", bufs=1) as constp,
        tc.tile_pool(name="wpool", bufs=1) as wp,
        tc.tile_pool(name="xpool", bufs=1) as xp,
        tc.tile_pool(name="qkvp", bufs=1) as qkvp,
        tc.tile_pool(name="aep", bufs=3) as aep,
        tc.tile_pool(name="outp", bufs=1) as outp,
        tc.tile_pool(name="normp", bufs=2) as normp,
        tc.tile_pool(name="sgp", bufs=3) as sgp,
        tc.tile_pool(name="pspair", bufs=2, space="PSUM") as pspair,
        tc.tile_pool(name="psav", bufs=2, space="PSUM") as psav,
        tc.tile_pool(name="pswo", bufs=2, space="PSUM") as pswo,
    ):
        # ---- persistent SBUF tensors ----
        bq_sb = constp.tile([P, 2], F32, name="bq_sb")
        bqs_sb = constp.tile([P, 2], F32, name="bqs_sb")
        bk_sb = constp.tile([P, 2], F32, name="bk_sb")
        bv1_sb = constp.tile([1, DG], F32, name="bv1_sb")
        bvb_sb = constp.tile([P, DG], F32, name="bvb_sb")

        wqt_sb = wp.tile([P, EC, DG], BF16, name="wqt_sb")
        wkt_sb = wp.tile([P, EC, DG], BF16, name="wkt_sb")
        wvt_sb = wp.tile([P, EC, DG], BF16, name="wvt_sb")
        wot_sb = wp.tile([P, 2, D], BF16, name="wot_sb")

        xt_sb = xp.tile([P, EC, S], BF16, name="xt_sb")

        qt_sb = qkvp.tile([P, 2, S], BF16, name="qt_sb")
        kt_sb = qkvp.tile([P, 2, S], BF16, name="kt_sb")
        v_sb = qkvp.tile([P, KC, NHG, DH + 1], BF16, name="v_sb")

        outt_sb = outp.tile([P, 2, S], BF16, name="outt_sb")

        # ---- constants ----
        for c in range(2):
            nc.sync.dma_start(bq_sb[:, c : c + 1], bq_d[c * P : (c + 1) * P, :])
            nc.sync.dma_start(bk_sb[:, c : c + 1], bk_d[c * P : (c + 1) * P, :])
        nc.scalar.mul(bqs_sb[:, :], bq_sb[:, :], 0.125)
        nc.sync.dma_start(bv1_sb[:, :], bv_d[:, :])
        nc.gpsimd.partition_broadcast(bvb_sb[:, :], bv1_sb[:, :], channels=P)
        # ones column in V for the fused softmax denominator
        nc.vector.memset(v_sb[:, :, :, DH : DH + 1], 1.0)

        # ---- input DMAs (xt+wqt interleaved: Q db0 streams on them) ----
        for ec in range(EC):
            nc.sync.dma_start(xt_sb[:, ec, :], xt_d[ec * P : (ec + 1) * P, :])
            nc.sync.dma_start(wqt_sb[:, ec, :], wqt_d[ec * P : (ec + 1) * P, :])
        for ec in range(EC):
            nc.sync.dma_start(wkt_sb[:, ec, :], wkt_d[ec * P : (ec + 1) * P, :])
        for ec in range(EC):
            nc.sync.dma_start(wvt_sb[:, ec, :], wvt_d[ec * P : (ec + 1) * P, :])
        for dc in range(2):
            nc.sync.dma_start(wot_sb[:, dc, :], wot_d[dc * P : (dc + 1) * P, :])

        # ---- QKV projections ----
        # QT/KT block for one db (128 d-dims = 2 heads): 2 parked pair tiles
        # (4 PSUM banks), ec-outer so the matmuls stream on the xt DMAs; one
        # bias+scale ACTIVATE per pair tile [128,1024].
        def qk_block(w_sb, dst_sb, db, bias_ap, scale):
            ps = [
                pspair.tile([P, W2], F32, name=f"pqk{pr}", tag="pp")
                for pr in range(2)
            ]
            for ec in range(EC):
                for pr in range(2):
                    for i in range(2):
                        nc.tensor.matmul(
                            ps[pr][:, i * QTW : (i + 1) * QTW],
                            w_sb[:, ec, db * P : (db + 1) * P],
                            xt_sb[:, ec, (2 * pr + i) * QTW : (2 * pr + i + 1) * QTW],
                            start=(ec == 0),
                            stop=(ec == EC - 1),
                        )
            for pr in range(2):
                nc.scalar.activation(
                    dst_sb[:, db, 2 * pr * QTW : (2 * pr + 2) * QTW],
                    ps[pr][:, :],
                    AF.Identity,
                    bias=bias_ap,
                    scale=scale,
                )

        # V natural for s-chunks (2p, 2p+1) in one pair tile; DVE adds the
        # (partition-broadcast) bias while casting to bf16 into v_sb.
        def v_pair(p):
            pv = pspair.tile([P, W2], F32, name="pv", tag="pp")
            for i in range(2):
                sc = 2 * p + i
                for ec in range(EC):
                    nc.tensor.matmul(
                        pv[:, i * QTW : i * QTW + DG],
                        xt_sb[:, ec, sc * P : (sc + 1) * P],
                        wvt_sb[:, ec, :],
                        start=(ec == 0),
                        stop=(ec == EC - 1),
                    )
            for i in range(2):
                nc.vector.tensor_add(
                    v_sb[:, 2 * p + i, :, 0:DH],
                    pv[:, i * QTW : i * QTW + DG].rearrange(
                        "p (h d) -> p h d", h=NHG
                    ),
                    bvb_sb[:, :].rearrange("p (h d) -> p h d", h=NHG),
                )

        # ---- attention: one head on one q tile, k-chunk pairs ----
        # ST pair p = chunks (2p, 2p+1) -> one [128,1024] PSUM pair tile ->
        # one Exp ACTIVATE -> (diagonal pairs only) gpsimd affine_select
        # zeroing the future entries of the bf16 exp tile -> two A@V matmuls
        # accumulating into this head's [65,512] PSUM accumulator.
        def attn_head(t, h):
            dc, po = divmod(h, 2)
            qoff = po * DH
            cmax = 4 * t + 4
            npairs = cmax // 2
            av = psav.tile([P, QTW], F32, name="av", tag="av")

            def emit_st_pair(p):
                stp = pspair.tile([P, W2], F32, name="stp", tag="pp")
                for i in range(2):
                    c = 2 * p + i
                    nc.tensor.matmul(
                        stp[:, i * QTW : (i + 1) * QTW],
                        kt_sb[qoff : qoff + DH, dc, c * P : (c + 1) * P],
                        qt_sb[qoff : qoff + DH, dc, t * QTW : (t + 1) * QTW],
                        start=True,
                        stop=True,
                    )
                return stp

            sts = {0: emit_st_pair(0)}
            if npairs > 1:
                sts[1] = emit_st_pair(1)
            for p in range(npairs):
                ae = aep.tile([P, W2], BF16, name="ae", tag="ae")
                nc.scalar.activation(ae[:, :], sts[p][:, :], AF.Exp)
                if 2 * p >= 4 * t:
                    # diagonal pair: keep ae[k, j, q] iff q >= 128*(j0+j) + k
                    j0 = 2 * p - 4 * t
                    nc.gpsimd.affine_select(
                        out=ae[:, :].rearrange("k (j q) -> k j q", j=2),
                        in_=ae[:, :].rearrange("k (j q) -> k j q", j=2),
                        compare_op=mybir.AluOpType.is_ge,
                        fill=0.0,
                        base=-128 * j0,
                        pattern=[[-128, 2], [1, QTW]],
                        channel_multiplier=-1,
                    )
                if p + 2 < npairs:
                    sts[p + 2] = emit_st_pair(p + 2)
                for i in range(2):
                    c = 2 * p + i
                    nc.tensor.matmul(
                        av[0 : DH + 1, :],
                        v_sb[:, c, h, :],
                        ae[:, i * QTW : (i + 1) * QTW],
                        start=(c == 0),
                        stop=(c == cmax - 1),
                    )
            norm_dispatch(h, t, av)

        def norm_dispatch(h, t, av):
            # av rows 0-63 = unnormalized out, row 64 = denominator.
            dc, po = divmod(h, 2)
            tq = slice(t * QTW, (t + 1) * QTW)
            rec = normp.tile([1, QTW], F32, name="rec", tag="rec")
            nc.vector.reciprocal(rec[:, :], av[DH : DH + 1, :])
            bc = normp.tile([DH, QTW], F32, name="bc", tag="bc")
            nc.gpsimd.partition_broadcast(bc[:, :], rec[:, :], channels=DH)
            if po == 0:
                nc.vector.tensor_mul(outt_sb[0:DH, dc, tq], av[0:DH, :], bc[:, :])
            else:
                odd = normp.tile([DH, QTW], BF16, name="odd", tag="odd")
                nc.vector.tensor_mul(odd[:, :], av[0:DH, :], bc[:, :])
                nc.sync.dma_start(outt_sb[DH:P, dc, tq], odd[:, :])

        def wo_tile(st4):
            # yT[:, q-tile st4] = sum_dc WoT_chunk.T @ outT_chunk; DVE
            # evacuates PSUM->SBUF, then DMA to DRAM.
            for mc in range(8):
                py = pswo.tile([P, QTW], F32, name="py", tag="wo")
                for dcw in range(2):
                    nc.tensor.matmul(
                        py[:, :],
                        wot_sb[:, dcw, mc * P : (mc + 1) * P],
                        outt_sb[:, dcw, st4 * QTW : (st4 + 1) * QTW],
                        start=(dcw == 0),
                        stop=(dcw == 1),
                    )
                sg = sgp.tile([P, QTW], F32, name="sg", tag="sg")
                nc.vector.tensor_copy(sg[:, :], py[:, :])
                nc.sync.dma_start(
                    yt_d[mc * P : (mc + 1) * P, st4 * QTW : (st4 + 1) * QTW],
                    sg[:, :],
                )

        # ---- main schedule ----
        # t=0 attention interleaved with the remaining projections so the
        # exp stream (ACT is the long pole) starts as early as possible.
        qk_block(wqt_sb, qt_sb, 0, bqs_sb[:, 0:1], 0.125)
        qk_block(wkt_sb, kt_sb, 0, bk_sb[:, 0:1], 1.0)
        v_pair(0)
        v_pair(1)
        attn_head(0, 0)
        v_pair(2)
        v_pair(3)
        attn_head(0, 1)
        qk_block(wqt_sb, qt_sb, 1, bqs_sb[:, 1:2], 0.125)
        qk_block(wkt_sb, kt_sb, 1, bk_sb[:, 1:2], 1.0)
        v_pair(4)
        v_pair(5)
        attn_head(0, 2)
        v_pair(6)
        v_pair(7)
        attn_head(0, 3)
        wo_tile(0)
        for t in range(1, NQT):
            for h in range(NHG):
                attn_head(t, h)
            wo_tile(t)


_NC_CACHE = None


def build_nc():
    global _NC_CACHE
    if _NC_CACHE is not None:
        return _NC_CACHE
    nc = bacc.Bacc("TRN2")
    xt = nc.dram_tensor("xt", [D, S], BF16, kind="ExternalInput")
    wqt = nc.dram_tensor("wqt", [D, DG], BF16, kind="ExternalInput")
    wkt = nc.dram_tensor("wkt", [D, DG], BF16, kind="ExternalInput")
    wvt = nc.dram_tensor("wvt", [D, DG], BF16, kind="ExternalInput")
    wot = nc.dram_tensor("wot", [DG, D], BF16, kind="ExternalInput")
    bq = nc.dram_tensor("bq", [DG, 1], F32, kind="ExternalInput")
    bk = nc.dram_tensor("bk", [DG, 1], F32, kind="ExternalInput")
    bv = nc.dram_tensor("bv", [1, DG], F32, kind="ExternalInput")
    yt = nc.dram_tensor("yt", [D, S], F32, kind="ExternalOutput")
    aps = tuple(h.ap() for h in (xt, wqt, wkt, wvt, wot, bq, bk, bv, yt))
    with tile.TileContext(nc) as tc:
        _emit(tc, aps)
    nc.finalize()
    _NC_CACHE = nc
    return nc


def make_in_maps(x, W_q, b_q, W_k, b_k, W_v, b_v, W_o):
    bf = ml_dtypes.bfloat16
    in_maps = []
    for core in range(8):
        b, hg = divmod(core, 4)
        sl = slice(hg * DG, (hg + 1) * DG)
        in_maps.append(
            {
                "xt": np.ascontiguousarray(np.asarray(x)[b].T.astype(bf)),
                "wqt": np.ascontiguousarray(np.asarray(W_q)[sl, :].T.astype(bf)),
                "wkt": np.ascontiguousarray(np.asarray(W_k)[sl, :].T.astype(bf)),
                "wvt": np.ascontiguousarray(np.asarray(W_v)[sl, :].T.astype(bf)),
                "wot": np.ascontiguousarray(np.asarray(W_o)[:, sl].T.astype(bf)),
                "bq": np.ascontiguousarray(
                    np.asarray(b_q)[sl].reshape(DG, 1), dtype=np.float32
                ),
                "bk": np.ascontiguousarray(
                    np.asarray(b_k)[sl].reshape(DG, 1), dtype=np.float32
                ),
                "bv": np.ascontiguousarray(
                    np.asarray(b_v)[sl].reshape(1, DG), dtype=np.float32
                ),
            }
        )
    return in_maps


def kernel(x, W_q, b_q, W_k, b_k, W_v, b_v, W_o, b_o, _trace=False):
    global LAST_EXEC_NS, LAST_RESULTS
    nc = build_nc()
    in_maps = make_in_maps(x, W_q, b_q, W_k, b_k, W_v, b_v, W_o)
    kw = {"trace": True} if _trace else {}
    res = run_bass_kernel_spmd(nc, in_maps, core_ids=list(range(8)), **kw)
    LAST_EXEC_NS = res.exec_time_ns
    LAST_RESULTS = res
    b_o = np.asarray(b_o, dtype=np.float32)
    out = np.empty((2, S, D), np.float32)
    for b in range(2):
        ysum = (
            res.results[4 * b]["yt"]
            + res.results[4 * b + 1]["yt"]
            + res.results[4 * b + 2]["yt"]
            + res.results[4 * b + 3]["yt"]
        )
        out[b] = ysum.T + b_o
    return out


# revision 9
# speedup vs baseline: 1.0870x; 1.0505x over previous
"""Causal self-attention Trainium2 kernel (8 NeuronCores).

Sharding: core = (batch b in {0,1}, head-group hg in {0..3}); each core owns
4 of the 16 heads (256 of the 1024 q/k/v dims) for one batch element.
Data parallel over batch, tensor parallel over heads; W_o is row-parallel so
each core emits a partial output that the host sums (+ b_o) at gather time.

Device dataflow (per core), everything in "transposed" layout so the
contraction dim always sits on SBUF partitions:
  xT [1024,2048] bf16, weights pre-transposed+cast on host.
  QT/KT [d=256, s=2048] bf16 (d on partitions, 2 blocks of 128)
  V natural [s, d] bf16 with a ones-column appended per head so the A@V
  matmul also produces the softmax denominator (row 64 of the PSUM tile).

Key perf structure vs the naive version:
  - Scores for two adjacent k-chunks of one head land in one 2-bank PSUM
    pair tile [128,1024]; ONE ACT Exp instruction covers both chunks
    (amortizes the ~352-cycle ACTIVATE fixed cost; ACT is the critical
    engine in the attention phase).
  - Causal masking happens AFTER the exp, as a gpsimd affine_select that
    zeroes the upper-triangle entries of the bf16 exp tile (exp can't
    overflow: scores are bounded by ~2 after the 1/8 scale). This keeps
    both DVE and ACT off the mask work.
  - t-outer / head-inner attention; the db=1 Q/K projection blocks and the
    V projection pairs are emitted interleaved into early t=0 attention so
    the exp stream starts as soon as Q/K db0 + V chunks 0-3 exist.
  - Normalization: direct DVE reciprocal on the denominator row (no DMA
    reshape round-trip), gpsimd partition_broadcast, one DVE multiply
    straight out of PSUM.
  - W_o projection per q-tile as soon as its last head is normalized;
    PSUM->SBUF evacuation on DVE (ACT stays reserved for exps).
"""

import sys

for _p in ("/opt/trn_rl_repo",):
    if _p not in sys.path:
        sys.path.insert(0, _p)

import numpy as np
import ml_dtypes

import concourse.bass as bass
import concourse.bacc as bacc
import concourse.mybir as mybir
from concourse import tile
from concourse.bass_utils import run_bass_kernel_spmd

P = 128
S = 2048  # sequence length
D = 1024  # d_model
DG = 256  # dims per head-group (4 heads x 64)
DH = 64   # head dim
NHG = 4   # heads per core
EC = D // P   # 8 contraction chunks over d_model
KC = S // P   # 16 key chunks
QTW = 512     # q tile width
NQT = S // QTW  # 4 q tiles
W2 = 2 * QTW  # pair-tile width (2 PSUM banks)
F32 = mybir.dt.float32
BF16 = mybir.dt.bfloat16
AF = mybir.ActivationFunctionType

LAST_EXEC_NS = None
LAST_RESULTS = None


def _emit(tc, aps):
    nc = tc.nc
    xt_d, wqt_d, wkt_d, wvt_d, wot_d, bq_d, bk_d, bv_d, yt_d = aps

    with (
        tc.tile_pool(name="const", bufs=1) as constp,
        tc.tile_pool(name="wpool", bufs=1) as wp,
        tc.tile_pool(name="xpool", bufs=1) as xp,
        tc.tile_pool(name="qkvp", bufs=1) as qkvp,
        tc.tile_pool(name="aep", bufs=3) as aep,
        tc.tile_pool(name="outp", bufs=1) as outp,
        tc.tile_pool(name="normp", bufs=2) as normp,
        tc.tile_pool(name="sgp", bufs=3) as sgp,
        tc.tile_pool(name="pspair", bufs=2, space="PSUM") as pspair,
        tc.tile_pool(name="psav", bufs=2, space="PSUM") as psav,
        tc.tile_pool(name="pswo", bufs=2, space="PSUM") as pswo,
    ):
        # ---- persistent SBUF tensors ----
        bq_sb = constp.tile([P, 2], F32, name="bq_sb")
        bqs_sb = constp.tile([P, 2], F32, name="bqs_sb")
        bk_sb = constp.tile([P, 2], F32, name="bk_sb")
        bv1_sb = constp.tile([1, DG], F32, name="bv1_sb")
        bvb_sb = constp.tile([P, DG], F32, name="bvb_sb")

        wqt_sb = wp.tile([P, EC, DG], BF16, name="wqt_sb")
        wkt_sb = wp.tile([P, EC, DG], BF16, name="wkt_sb")
        wvt_sb = wp.tile([P, EC, DG], BF16, name="wvt_sb")
        wot_sb = wp.tile([P, 2, D], BF16, name="wot_sb")

        xt_sb = xp.tile([P, EC, S], BF16, name="xt_sb")

        qt_sb = qkvp.tile([P, 2, S], BF16, name="qt_sb")
        kt_sb = qkvp.tile([P, 2, S], BF16, name="kt_sb")
        v_sb = qkvp.tile([P, KC, NHG, DH + 1], BF16, name="v_sb")

        outt_sb = outp.tile([P, 2, S], BF16, name="outt_sb")

        # ---- constants ----
        for c in range(2):
            nc.sync.dma_start(bq_sb[:, c : c + 1], bq_d[c * P : (c + 1) * P, :])
            nc.sync.dma_start(bk_sb[:, c : c + 1], bk_d[c * P : (c + 1) * P, :])
        nc.scalar.mul(bqs_sb[:, :], bq_sb[:, :], 0.125)
        nc.sync.dma_start(bv1_sb[:, :], bv_d[:, :])
        nc.gpsimd.partition_broadcast(bvb_sb[:, :], bv1_sb[:, :], channels=P)
        # ones column in V for the fused softmax denominator
        nc.vector.memset(v_sb[:, :, :, DH : DH + 1], 1.0)

        # ---- input DMAs ----
        # xt chunks stream on the sync queue; all weight chunks issue in
        # parallel from the gpsimd queue so Q/K/V matmuls are gated only by
        # the xt stream, not by DMA-issue serialization.
        for ec in range(EC):
            nc.sync.dma_start(xt_sb[:, ec, :], xt_d[ec * P : (ec + 1) * P, :])
        for ec in range(EC):
            nc.gpsimd.dma_start(wqt_sb[:, ec, :], wqt_d[ec * P : (ec + 1) * P, :])
        for ec in range(EC):
            nc.gpsimd.dma_start(wkt_sb[:, ec, :], wkt_d[ec * P : (ec + 1) * P, :])
        for ec in range(EC):
            nc.gpsimd.dma_start(wvt_sb[:, ec, :], wvt_d[ec * P : (ec + 1) * P, :])
        for dc in range(2):
            nc.gpsimd.dma_start(wot_sb[:, dc, :], wot_d[dc * P : (dc + 1) * P, :])

        # ---- QKV projections ----
        # QT/KT block for one db (128 d-dims = 2 heads): 2 parked pair tiles
        # (4 PSUM banks), ec-outer so the matmuls stream on the xt DMAs; one
        # bias+scale ACTIVATE per pair tile [128,1024].
        def qk_block(w_sb, dst_sb, db, bias_ap, scale):
            ps = [
                pspair.tile([P, W2], F32, name=f"pqk{pr}", tag="pp")
                for pr in range(2)
            ]
            for ec in range(EC):
                for pr in range(2):
                    for i in range(2):
                        nc.tensor.matmul(
                            ps[pr][:, i * QTW : (i + 1) * QTW],
                            w_sb[:, ec, db * P : (db + 1) * P],
                            xt_sb[:, ec, (2 * pr + i) * QTW : (2 * pr + i + 1) * QTW],
                            start=(ec == 0),
                            stop=(ec == EC - 1),
                        )
            for pr in range(2):
                nc.scalar.activation(
                    dst_sb[:, db, 2 * pr * QTW : (2 * pr + 2) * QTW],
                    ps[pr][:, :],
                    AF.Identity,
                    bias=bias_ap,
                    scale=scale,
                )

        # V natural for s-chunks (2p, 2p+1) in one pair tile; DVE adds the
        # (partition-broadcast) bias while casting to bf16 into v_sb.
        def v_pair(p):
            pv = pspair.tile([P, W2], F32, name="pv", tag="pp")
            for i in range(2):
                sc = 2 * p + i
                for ec in range(EC):
                    nc.tensor.matmul(
                        pv[:, i * QTW : i * QTW + DG],
                        xt_sb[:, ec, sc * P : (sc + 1) * P],
                        wvt_sb[:, ec, :],
                        start=(ec == 0),
                        stop=(ec == EC - 1),
                    )
            for i in range(2):
                nc.vector.tensor_add(
                    v_sb[:, 2 * p + i, :, 0:DH],
                    pv[:, i * QTW : i * QTW + DG].rearrange(
                        "p (h d) -> p h d", h=NHG
                    ),
                    bvb_sb[:, :].rearrange("p (h d) -> p h d", h=NHG),
                )

        # ---- attention: one head on one q tile, k-chunk pairs ----
        # ST pair p = chunks (2p, 2p+1) -> one [128,1024] PSUM pair tile ->
        # one Exp ACTIVATE -> (diagonal pairs only) gpsimd affine_select
        # zeroing the future entries of the bf16 exp tile -> two A@V matmuls
        # accumulating into this head's [65,512] PSUM accumulator.
        def attn_head(t, h):
            dc, po = divmod(h, 2)
            qoff = po * DH
            cmax = 4 * t + 4
            npairs = cmax // 2
            av = psav.tile([P, QTW], F32, name="av", tag="av")

            def emit_st_pair(p, filler=False):
                stp = pspair.tile([P, W2], F32, name="stp", tag="pp")
                if filler:
                    # Duplicate ST matmul into the half the real ST overwrites
                    # (its start=True discards this). Pure PE filler: keeps the
                    # MM stream dense enough that the HAM clock monitor holds
                    # the PE at full rate through the ACT-paced exp phase.
                    nc.tensor.matmul(
                        stp[:, 0:QTW],
                        kt_sb[qoff : qoff + DH, dc, 2 * p * P : (2 * p + 1) * P],
                        qt_sb[qoff : qoff + DH, dc, t * QTW : (t + 1) * QTW],
                        start=True,
                        stop=True,
                    )
                for i in range(2):
                    c = 2 * p + i
                    nc.tensor.matmul(
                        stp[:, i * QTW : (i + 1) * QTW],
                        kt_sb[qoff : qoff + DH, dc, c * P : (c + 1) * P],
                        qt_sb[qoff : qoff + DH, dc, t * QTW : (t + 1) * QTW],
                        start=True,
                        stop=True,
                    )
                return stp

            sts = {0: emit_st_pair(0)}
            if npairs > 1:
                sts[1] = emit_st_pair(1)
            for p in range(npairs):
                ae = aep.tile([P, W2], BF16, name="ae", tag="ae")
                nc.scalar.activation(ae[:, :], sts[p][:, :], AF.Exp)
                if 2 * p >= 4 * t:
                    # diagonal pair: keep ae[k, j, q] iff q >= 128*(j0+j) + k
                    j0 = 2 * p - 4 * t
                    nc.gpsimd.affine_select(
                        out=ae[:, :].rearrange("k (j q) -> k j q", j=2),
                        in_=ae[:, :].rearrange("k (j q) -> k j q", j=2),
                        compare_op=mybir.AluOpType.is_ge,
                        fill=0.0,
                        base=-128 * j0,
                        pattern=[[-128, 2], [1, QTW]],
                        channel_multiplier=-1,
                    )
                if p + 2 < npairs:
                    sts[p + 2] = emit_st_pair(p + 2, filler=True)
                for i in range(2):
                    c = 2 * p + i
                    nc.tensor.matmul(
                        av[0 : DH + 1, :],
                        v_sb[:, c, h, :],
                        ae[:, i * QTW : (i + 1) * QTW],
                        start=(c == 0),
                        stop=(c == cmax - 1),
                    )
            norm_dispatch(h, t, av)

        def norm_dispatch(h, t, av):
            # av rows 0-63 = unnormalized out, row 64 = denominator.
            # Reciprocal is an iterative 8-cycle/elem DVE op, so run it with
            # all 128 lanes via a DMA reshape [1,512]<->[128,4]; the whole
            # chain is off the critical path (next head's pairs keep PE/ACT
            # fed while it drains).
            dc, po = divmod(h, 2)
            tq = slice(t * QTW, (t + 1) * QTW)
            den = normp.tile([1, QTW], F32, name="den", tag="den")
            nc.vector.tensor_copy(den[:, :], av[DH : DH + 1, :])
            denP = normp.tile([P, 4], F32, name="denP", tag="denP")
            nc.gpsimd.dma_start(denP[:, :], den[:, :])
            recP = normp.tile([P, 4], F32, name="recP", tag="recP")
            nc.vector.reciprocal(recP[:, :], denP[:, :])
            rec = normp.tile([1, QTW], F32, name="rec", tag="rec")
            nc.gpsimd.dma_start(rec[:, :], recP[:, :])
            bc = normp.tile([DH, QTW], F32, name="bc", tag="bc")
            nc.gpsimd.partition_broadcast(bc[:, :], rec[:, :], channels=DH)
            if po == 0:
                nc.vector.tensor_mul(outt_sb[0:DH, dc, tq], av[0:DH, :], bc[:, :])
            else:
                odd = normp.tile([DH, QTW], BF16, name="odd", tag="odd")
                nc.vector.tensor_mul(odd[:, :], av[0:DH, :], bc[:, :])
                nc.gpsimd.dma_start(outt_sb[DH:P, dc, tq], odd[:, :])

        def wo_tile(st4):
            # yT[:, q-tile st4] = sum_dc WoT_chunk.T @ outT_chunk; DVE
            # evacuates PSUM->SBUF, then DMA to DRAM.
            for mc in range(8):
                py = pswo.tile([P, QTW], F32, name="py", tag="wo")
                for dcw in range(2):
                    nc.tensor.matmul(
                        py[:, :],
                        wot_sb[:, dcw, mc * P : (mc + 1) * P],
                        outt_sb[:, dcw, st4 * QTW : (st4 + 1) * QTW],
                        start=(dcw == 0),
                        stop=(dcw == 1),
                    )
                sg = sgp.tile([P, QTW], F32, name="sg", tag="sg")
                nc.vector.tensor_copy(sg[:, :], py[:, :])
                nc.sync.dma_start(
                    yt_d[mc * P : (mc + 1) * P, st4 * QTW : (st4 + 1) * QTW],
                    sg[:, :],
                )

        # ---- main schedule ----
        # t=0 attention interleaved with the remaining projections so the
        # exp stream (ACT is the long pole) starts as early as possible.
        qk_block(wqt_sb, qt_sb, 0, bqs_sb[:, 0:1], 0.125)
        qk_block(wkt_sb, kt_sb, 0, bk_sb[:, 0:1], 1.0)
        v_pair(0)
        v_pair(1)
        attn_head(0, 0)
        v_pair(2)
        v_pair(3)
        attn_head(0, 1)
        qk_block(wqt_sb, qt_sb, 1, bqs_sb[:, 1:2], 0.125)
        qk_block(wkt_sb, kt_sb, 1, bk_sb[:, 1:2], 1.0)
        v_pair(4)
        v_pair(5)
        attn_head(0, 2)
        v_pair(6)
        v_pair(7)
        attn_head(0, 3)
        # wo_tile(t) is emitted one head-block late so the PE has pair work
        # in flight while tile t's norm chains drain.
        for t in range(1, NQT):
            attn_head(t, 0)
            wo_tile(t - 1)
            for h in range(1, NHG):
                attn_head(t, h)
        wo_tile(NQT - 1)


_NC_CACHE = None


def build_nc():
    global _NC_CACHE
    if _NC_CACHE is not None:
        return _NC_CACHE
    nc = bacc.Bacc("TRN2")
    xt = nc.dram_tensor("xt", [D, S], BF16, kind="ExternalInput")
    wqt = nc.dram_tensor("wqt", [D, DG], BF16, kind="ExternalInput")
    wkt = nc.dram_tensor("wkt", [D, DG], BF16, kind="ExternalInput")
    wvt = nc.dram_tensor("wvt", [D, DG], BF16, kind="ExternalInput")
    wot = nc.dram_tensor("wot", [DG, D], BF16, kind="ExternalInput")
    bq = nc.dram_tensor("bq", [DG, 1], F32, kind="ExternalInput")
    bk = nc.dram_tensor("bk", [DG, 1], F32, kind="ExternalInput")
    bv = nc.dram_tensor("bv", [1, DG], F32, kind="ExternalInput")
    yt = nc.dram_tensor("yt", [D, S], F32, kind="ExternalOutput")
    aps = tuple(h.ap() for h in (xt, wqt, wkt, wvt, wot, bq, bk, bv, yt))
    with tile.TileContext(nc) as tc:
        _emit(tc, aps)
    nc.finalize()
    _NC_CACHE = nc
    return nc


def make_in_maps(x, W_q, b_q, W_k, b_k, W_v, b_v, W_o):
    bf = ml_dtypes.bfloat16
    in_maps = []
    for core in range(8):
        b, hg = divmod(core, 4)
        sl = slice(hg * DG, (hg + 1) * DG)
        in_maps.append(
            {
                "xt": np.ascontiguousarray(np.asarray(x)[b].T.astype(bf)),
                "wqt": np.ascontiguousarray(np.asarray(W_q)[sl, :].T.astype(bf)),
                "wkt": np.ascontiguousarray(np.asarray(W_k)[sl, :].T.astype(bf)),
                "wvt": np.ascontiguousarray(np.asarray(W_v)[sl, :].T.astype(bf)),
                "wot": np.ascontiguousarray(np.asarray(W_o)[:, sl].T.astype(bf)),
                "bq": np.ascontiguousarray(
                    np.asarray(b_q)[sl].reshape(DG, 1), dtype=np.float32
                ),
                "bk": np.ascontiguousarray(
                    np.asarray(b_k)[sl].reshape(DG, 1), dtype=np.float32
                ),
                "bv": np.ascontiguousarray(
                    np.asarray(b_v)[sl].reshape(1, DG), dtype=np.float32
                ),
            }
        )
    return in_maps


def kernel(x, W_q, b_q, W_k, b_k, W_v, b_v, W_o, b_o, _trace=False):
    global LAST_EXEC_NS, LAST_RESULTS
    nc = build_nc()
    in_maps = make_in_maps(x, W_q, b_q, W_k, b_k, W_v, b_v, W_o)
    kw = {"trace": True} if _trace else {}
    res = run_bass_kernel_spmd(nc, in_maps, core_ids=list(range(8)), **kw)
    LAST_EXEC_NS = res.exec_time_ns
    LAST_RESULTS = res
    b_o = np.asarray(b_o, dtype=np.float32)
    out = np.empty((2, S, D), np.float32)
    for b in range(2):
        ysum = (
            res.results[4 * b]["yt"]
            + res.results[4 * b + 1]["yt"]
            + res.results[4 * b + 2]["yt"]
            + res.results[4 * b + 3]["yt"]
        )
        out[b] = ysum.T + b_o
    return out
